# revision 25
# baseline (speedup 1.0000x reference)
"""Multi-head attention (2D-RoPE, masked softmax) on 8 Trainium2 NeuronCores.

Sharding: 4 head-groups (3 heads each) x 2 query-halves (1160 rows each).
Each core computes full attention for its 3 heads over its 1160 query rows
against all 2320 keys, plus its share of the output projection; the host
sums the 8 partial projections and adds the (folded) biases.

v3 notes:
  - xt ships as 3 separate column-slice tensors (512 / 768 / 1040 cols)
    so every DMA is 128 contiguous multi-KB runs (line rate, not
    descriptor-bound) and phase A starts on slice A at ~11us.
  - No memsets / zero padding.  Score matmuls run as row-tiled concurrent
    pairs (chunk i on PE rows 0-63, chunk i+1 on rows 64-127) against
    kt/qt tiles whose upper 64 partitions duplicate the lower 64
    (per-tile dup copies emitted right after each rope tile).
  - Scores group as quad/pair/quad/... so one PSUM [128,2048] (4 banks) +
    one [128,1024] (2 banks) alternate: exp ACTIVATEs cover 4 chunks at a
    time where possible (63 instead of 90 exp instructions).
  - Head 2's K projection col-tiles with its Q projection (shared moving
    operand, separate PSUM banks - start=True zeroes a whole bank);
    heads 0/1 K and Q project as single M=128 matmuls with one stacked
    [128,n] bias Identity each.
  - Rope runs interleaved into the phase-A job stream (Vector work hides
    under projection matmuls); output projection contracts heads 0+1 as
    one K=128 matmul plus a K=64 matmul for head 2; output stores bf16.
  - finish chain: one [65, ln] PSUM->SBUF copy grabs ctx + Z together,
    reciprocal_approx_fast, normalize multiply reads broadcast PSUM row
    directly; chain + projection slices deferred into the next l-tile's
    instruction stream so the PE never idles at tile boundaries.
  - V-bias and output bias never touch the device:
    out = softmax(..) @ (Vx + bv) @ Wp.T + bp = dev_out + (Wp @ bv + bp).
"""
import sys
if '/opt/trn_rl_repo' not in sys.path:
    sys.path.insert(0, '/opt/trn_rl_repo')
import numpy as np

SEQ, E, NH, D = 2320, 768, 12, 64
GRID, TASK = 48, 16
SQ = SEQ // 2           # query rows per core
HG = 3                  # heads per core
SCALE = D ** -0.5
EC = 6                  # embed chunks of 128
L_TILES = [(0, 512), (512, 392), (904, 256)]
MC = [(i * 128, min(128, SEQ - i * 128)) for i in range(19)]
PT = [(i * 128, min(128, SQ - i * 128)) for i in range(10)]
XA, XB = 512, 1280      # xt slice boundaries: A=[0,512) B=[512,1280) C=[1280,2320)
GROUPS = [(0, 1, 2, 3), (4, 5), (6, 7, 8, 9), (10, 11),
          (12, 13, 14, 15), (16, 17), (18,)]
G_TILE = ['Q', 'P', 'Q', 'P', 'Q', 'P', 'S']  # psum tile per group

ROWTILE_SCORES = True   # concurrent row-tiled score pairs (kt/qt row dup)
COLTILE_KQ2 = True      # head2 K col-tiled with head2 Q
DEBUG_DUMP = False       # dump intermediate tensors as extra outputs

_prog = None


def _build():
    import concourse.mybir as mybir
    import concourse.tile as tile
    from concourse import bacc

    F32, F32R = mybir.dt.float32, mybir.dt.float32r
    BF16 = mybir.dt.bfloat16
    AF = mybir.ActivationFunctionType

    nc = bacc.Bacc('TRN2', target_bir_lowering=False, debug=False, num_devices=8)
    dp = nc.declare_dram_parameter
    xta_d = dp("xta", [128, EC, XA], BF16, isOutput=False)
    xtb_d = dp("xtb", [128, EC, XB - XA], BF16, isOutput=False)
    xtc_d = dp("xtc", [128, EC, SEQ - XB], BF16, isOutput=False)
    wk_d = dp("wk", [128, EC, 192], BF16, isOutput=False)
    wq_d = dp("wq", [128, EC, 192], BF16, isOutput=False)
    wv_d = dp("wv", [128, EC, 192], BF16, isOutput=False)
    wp1_d = dp("wp1", [128, E], F32R, isOutput=False)
    wp2_d = dp("wp2", [64, E], F32R, isOutput=False)
    b_d = dp("b", [128, 3], F32, isOutput=False)
    mk_d = dp("mk", [128, 19], F32, isOutput=False)
    ck_d = dp("ck", [64, SEQ], BF16, isOutput=False)
    sk_d = dp("sk", [64, SEQ], BF16, isOutput=False)
    cq_d = dp("cq", [64, SQ], BF16, isOutput=False)
    sq_d = dp("sq", [64, SQ], BF16, isOutput=False)
    out_d = dp("pout", [SQ, E], BF16, isOutput=True)
    if DEBUG_DUMP:
        dbg_rk_d = dp("dbg_rk01", [128, SEQ], BF16, isOutput=True)
        dbg_kt_d = dp("dbg_kt0", [64, SEQ], BF16, isOutput=True)
        dbg_v_d = dp("dbg_vall", [128, 19 * HG * 65], BF16, isOutput=True)
        dbg_c_d = dp("dbg_ctx", [128, SQ], F32, isOutput=True)

    with tile.TileContext(nc) as tc:
        with (
            tc.tile_pool(name="long", bufs=1) as lp,
            tc.tile_pool(name="zp", bufs=2) as zp,
        ):
            xta = lp.tile([128, EC, XA], BF16, tag="xta")
            xtb = lp.tile([128, EC, XB - XA], BF16, tag="xtb")
            xtc = lp.tile([128, EC, SEQ - XB], BF16, tag="xtc")
            wk_sb = lp.tile([128, EC, 192], BF16, tag="wk")
            wq_sb = lp.tile([128, EC, 192], BF16, tag="wq")
            wv_sb = lp.tile([128, EC, 192], BF16, tag="wv")
            wp1_sb = lp.tile([128, E], F32R, tag="wp1")
            wp2_sb = lp.tile([64, E], F32R, tag="wp2")
            b_sb = lp.tile([128, 3], F32, tag="b")
            mk_sb = lp.tile([128, 19], F32, tag="mk")
            ck_sb = lp.tile([128, SEQ], BF16, tag="ck")
            sk_sb = lp.tile([128, SEQ], BF16, tag="sk")
            cq_sb = lp.tile([128, SQ], BF16, tag="cq")
            sq_sb = lp.tile([128, SQ], BF16, tag="sq")
            rk01 = lp.tile([128, SEQ], BF16, tag="rk01")
            rq01 = lp.tile([128, SQ], BF16, tag="rq01")
            raw2 = lp.tile([128, SEQ], BF16, tag="raw2")
            kt_h = [lp.tile([128, SEQ], BF16, tag=f"kt{h}", name=f"kt{h}")
                    for h in range(HG)]
            qt_h = [lp.tile([128, SQ], BF16, tag=f"qt{h}", name=f"qt{h}")
                    for h in range(HG)]
            v_all = lp.tile([128, 19, HG, 65], BF16, tag="v_all")
            ctx01 = lp.tile([128, SQ], F32R, tag="ctx01")
            ctx2 = lp.tile([64, SQ], F32R, tag="ctx2")
            ones64 = lp.tile([1, 64], F32R, tag="ones64")

            nc.gpsimd.memset(ones64[:].bitcast(F32), 1.0)

            # ---- input DMAs: few, large, spread across issue paths ----
            nc.gpsimd.dma_start(wk_sb[:], wk_d[:])
            nc.gpsimd.dma_start(b_sb[:], b_d[:])
            nc.gpsimd.dma_start(wq_sb[:], wq_d[:])
            nc.gpsimd.dma_start(wv_sb[:], wv_d[:])
            nc.gpsimd.dma_start(mk_sb[:], mk_d[:])
            nc.sync.dma_start(xta[:], xta_d[:])
            nc.sync.dma_start(xtb[:], xtb_d[:])
            nc.sync.dma_start(xtc[:], xtc_d[:])
            nc.sync.dma_start(wp1_sb[:], wp1_d[:])
            nc.sync.dma_start(wp2_sb[:], wp2_d[:])
            nc.gpsimd.dma_start(ck_sb[0:64, :], ck_d[:])
            nc.gpsimd.dma_start(sk_sb[0:64, :], sk_d[:])
            nc.gpsimd.dma_start(cq_sb[0:64, :], cq_d[:])
            nc.gpsimd.dma_start(sq_sb[0:64, :], sq_d[:])
            nc.vector.tensor_copy(ck_sb[64:128, :], ck_sb[0:64, :])
            nc.vector.tensor_copy(sk_sb[64:128, :], sk_sb[0:64, :])
            nc.vector.tensor_copy(cq_sb[64:128, :], cq_sb[0:64, :])
            nc.vector.tensor_copy(sq_sb[64:128, :], sq_sb[0:64, :])
            # softmax-denominator ones column for all chunks at once
            nc.vector.tensor_copy(
                v_all[:, :, :, 64:65],
                mk_sb[:, :].to_broadcast([128, 19, HG, 1]))

            def xt_of(off, n):
                if off + n <= XA:
                    return xta, off
                if off + n <= XB:
                    return xtb, off - XA
                return xtc, off - XB

            # ---- phase A: QKV projections (+ interleaved rope) ----
            with tc.tile_pool(name="pk", bufs=3, space="PSUM") as pkp, \
                 tc.tile_pool(name="pv", bufs=2, space="PSUM") as pvp:

                def v_tile(i):
                    # mask folded into xt on the host (zeroed columns), so
                    # the PSUM->SBUF move runs on ScalarE, not Vector
                    off, m = MC[i]
                    xt, lo = xt_of(off, m)
                    pv = pvp.tile([128, 192], F32, tag="pv", name="pv")
                    for c in range(EC):
                        nc.tensor.matmul(
                            pv[0:m, :], xt[:, c, lo:lo + m], wv_sb[:, c, :],
                            start=(c == 0), stop=(c == EC - 1))
                    nc.scalar.activation(
                        v_all[0:m, i, :, 0:64],
                        pv[0:m, 0:192].rearrange("p (h d) -> p h d", h=HG),
                        AF.Identity, bias=0.0, scale=1.0)

                def mm128(w_sb, bcol, rawt, off, n):
                    xt, lo = xt_of(off, n)
                    ps = pkp.tile([128, 512], F32, tag="pk", name="pk")
                    for c in range(EC):
                        nc.tensor.matmul(
                            ps[0:128, 0:n], w_sb[:, c, 0:128], xt[:, c, lo:lo + n],
                            start=(c == 0), stop=(c == EC - 1))
                    nc.scalar.activation(
                        rawt[0:128, off:off + n], ps[0:128, 0:n], AF.Identity,
                        bias=b_sb[:, bcol:bcol + 1], scale=1.0)

                def kq2(off, n):
                    # head2 K and Q over the query range; when col-tiled the
                    # two chains share the moving xt operand but use separate
                    # PSUM banks (start=True zeroes a whole bank).
                    xt, lo = xt_of(off, n)
                    psa = pkp.tile([128, 512], F32, tag="pk", name="psa")
                    psb = pkp.tile([128, 512], F32, tag="pk", name="psb")
                    qrow = 64 if COLTILE_KQ2 else 0
                    for c in range(EC):
                        nc.tensor.matmul(
                            psa[0:64, 0:n], wk_sb[:, c, 128:192], xt[:, c, lo:lo + n],
                            start=(c == 0), stop=(c == EC - 1))
                        nc.tensor.matmul(
                            psb[qrow:qrow + 64, 0:n], wq_sb[:, c, 128:192],
                            xt[:, c, lo:lo + n],
                            start=(c == 0), stop=(c == EC - 1))
                    nc.scalar.activation(
                        raw2[0:64, off:off + n], psa[0:64, 0:n], AF.Identity,
                        bias=b_sb[0:64, 2:3], scale=1.0)
                    nc.scalar.activation(
                        raw2[64:128, off:off + n], psb[qrow:qrow + 64, 0:n],
                        AF.Identity, bias=b_sb[64:128, 2:3], scale=1.0)

                def k2a(off, n):
                    xt, lo = xt_of(off, n)
                    ps = pkp.tile([128, 512], F32, tag="pk", name="pk")
                    for c in range(EC):
                        nc.tensor.matmul(
                            ps[0:64, 0:n], wk_sb[:, c, 128:192], xt[:, c, lo:lo + n],
                            start=(c == 0), stop=(c == EC - 1))
                    nc.scalar.activation(
                        raw2[0:64, off:off + n], ps[0:64, 0:n], AF.Identity,
                        bias=b_sb[0:64, 2:3], scale=1.0)

                # ---- rope (SBUF-only; interleaves with projection PE work)
                def rope1(rawt, hb, cos_sb, sin_sb, off, n, outt):
                    t1 = zp.tile([64, 512], BF16, tag="rt1", name="rt1", bufs=3)
                    t2 = zp.tile([64, 512], BF16, tag="rt2", name="rt2", bufs=3)
                    nc.vector.tensor_mul(
                        t1[0:64, 0:n], rawt[hb:hb + 64, off:off + n],
                        cos_sb[hb:hb + 64, off:off + n])
                    for bi in range(2):
                        src = hb + bi * 32 + (32 if bi % 2 == 0 else -32)
                        nc.vector.tensor_mul(
                            t2[bi * 32:(bi + 1) * 32, 0:n],
                            rawt[src:src + 32, off:off + n],
                            sin_sb[src:src + 32, off:off + n])
                    nc.vector.tensor_add(
                        outt[0:64, off:off + n], t1[0:64, 0:n], t2[0:64, 0:n])
                    if ROWTILE_SCORES:
                        # duplicate rows for the row-tiled score pairs
                        nc.vector.tensor_copy(
                            outt[64:128, off:off + n], outt[0:64, off:off + n])

                def ropek01(off, n):
                    rope1(rk01, 0, ck_sb, sk_sb, off, n, kt_h[0])
                    rope1(rk01, 64, ck_sb, sk_sb, off, n, kt_h[1])

                def ropeq01(off, n):
                    rope1(rq01, 0, cq_sb, sq_sb, off, n, qt_h[0])
                    rope1(rq01, 64, cq_sb, sq_sb, off, n, qt_h[1])

                jobs = [
                    ("k01", wk_d, (0, 512)), ("rk01", (0, 512)),
                    ("kq2", (0, 512)), ("rk2", (0, 512)), ("rq2", (0, 512)),
                    ("q01", wq_d, (0, 512)), ("rq01", (0, 512)),
                    ("v", 0), ("v", 1), ("v", 2), ("v", 3),
                    ("k01", wk_d, (512, 512)), ("rk01", (512, 512)),
                    ("kq2", (512, 392)),
                    ("q01", wq_d, (512, 392)), ("rq01", (512, 392)),
                    ("v", 4), ("v", 5),
                    ("k01", wk_d, (1024, 256)),
                    ("kq2", (904, 256)), ("rk2", (512, 512)),
                    ("rq2", (512, 392)), ("rq2", (904, 256)),
                    ("q01", wq_d, (904, 256)), ("rq01", (904, 256)),
                    ("k2a", (1160, 120)),
                    ("v", 6), ("v", 7), ("v", 8), ("v", 9),
                    ("k01", wk_d, (1280, 512)), ("rk01", (1024, 512)),
                    ("k2a", (1280, 512)), ("rk2", (1024, 512)),
                    ("v", 10), ("v", 11), ("v", 12),
                    ("k01", wk_d, (1792, 512)), ("rk01", (1536, 512)),
                    ("k2a", (1792, 512)), ("rk2", (1536, 512)),
                    ("v", 13), ("v", 14), ("v", 15),
                    ("k01", wk_d, (2304, 16)), ("rk01", (2048, 272)),
                    ("k2a", (2304, 16)), ("rk2", (2048, 272)),
                    ("v", 16), ("v", 17), ("v", 18),
                ]
                for job in jobs:
                    kind = job[0]
                    if kind == "v":
                        v_tile(job[1])
                    elif kind == "k01":
                        mm128(wk_sb, 0, rk01, *job[2])
                    elif kind == "q01":
                        mm128(wq_sb, 1, rq01, *job[2])
                    elif kind == "kq2":
                        kq2(*job[1])
                    elif kind == "k2a":
                        k2a(*job[1])
                    elif kind == "rk01":
                        ropek01(*job[1])
                    elif kind == "rq01":
                        ropeq01(*job[1])
                    elif kind == "rk2":
                        rope1(raw2, 0, ck_sb, sk_sb, *job[1], kt_h[2])
                    elif kind == "rq2":
                        rope1(raw2, 64, cq_sb, sq_sb, *job[1], qt_h[2])

            # ---- attention ----
            with tc.tile_pool(name="ep", bufs=3) as ep, \
                 tc.tile_pool(name="op", bufs=2) as op, \
                 tc.tile_pool(name="rzp", bufs=2) as rzp, \
                 tc.tile_pool(name="psQ", bufs=1, space="PSUM") as psQ, \
                 tc.tile_pool(name="psP", bufs=1, space="PSUM") as psP, \
                 tc.tile_pool(name="pc3", bufs=1, space="PSUM") as pc3, \
                 tc.tile_pool(name="pp3", bufs=1, space="PSUM") as pp3:
                PROJ_OF_LT = {0: PT[0:4], 1: PT[4:7], 2: PT[7:10]}
                CTX_OF_H = [(ctx01, 0), (ctx01, 64), (ctx2, 0)]

                def proj_slice(toff, tm):
                    outsb = op.tile([128, E], BF16, tag="outsb", name="outsb")
                    for half in range(2):
                        hs = half * 384
                        pp = pp3.tile([128, 512], F32, tag="pp", name="pp")
                        nc.tensor.matmul(
                            pp[0:tm, 0:384], ctx01[0:128, toff:toff + tm],
                            wp1_sb[0:128, hs:hs + 384], start=True, stop=False)
                        nc.tensor.matmul(
                            pp[0:tm, 0:384], ctx2[0:64, toff:toff + tm],
                            wp2_sb[0:64, hs:hs + 384], start=False, stop=True)
                        nc.vector.tensor_copy(outsb[0:tm, hs:hs + 384], pp[0:tm, 0:384])
                    nc.sync.dma_start(out_d[toff:toff + tm, :], outsb[0:tm, :])

                pending = []       # deferred normalize jobs
                proj_ready = []    # proj slices whose ctx inputs are complete

                def finish_tile(z):
                    # NOTE: custom DVE ops (reciprocal_approx_*) must read
                    # and write at the SAME partition base on hardware - the
                    # Z row is parked at base 0 (zrow) before the recip.
                    zrow, ctxu, ctxap, hb, loff2, ln2 = z
                    rzf = zp.tile([1, 512], F32, tag="rzf", name="rzf")
                    nc.vector.reciprocal_approx_fast(rzf[0:1, 0:ln2], zrow[0:1, 0:ln2])
                    rzr = zp.tile([1, 512], F32R, tag="rzr", name="rzr")
                    nc.vector.tensor_copy(rzr[0:1, 0:ln2], rzf[0:1, 0:ln2])
                    przb = pp3.tile([128, 512], F32, tag="pp", name="przb")
                    nc.tensor.matmul(
                        przb[0:64, 0:ln2], ones64[:], rzr[0:1, 0:ln2],
                        start=True, stop=True)
                    nc.vector.tensor_mul(
                        ctxap[hb:hb + 64, loff2:loff2 + ln2],
                        ctxu[0:64, 0:ln2], przb[0:64, 0:ln2])

                def pop_finish():
                    z = pending.pop(0)
                    finish_tile(z)
                    if z[2] is ctx2:
                        proj_ready.extend(PROJ_OF_LT[L_TILES.index((z[4], z[5]))])

                for lt_i, (loff, ln) in enumerate(L_TILES):
                    for h in range(HG):
                        ktap, qtap = kt_h[h], qt_h[h]
                        pctx = pc3.tile([65, 512], F32, tag="pctx")
                        exs = {}

                        def scores_exp(g):
                            chunks = GROUPS[g]
                            if G_TILE[g] == 'Q':
                                ps = psQ.tile([128, 2048], F32, tag="psq", name="psq")
                            elif G_TILE[g] == 'P':
                                ps = psP.tile([128, 1024], F32, tag="psp", name="psp")
                            else:
                                ps = pp3.tile([128, 512], F32, tag="pp", name="pss")
                            for j, i in enumerate(chunks):
                                moff, m = MC[i]
                                hb = 64 * (j % 2) if ROWTILE_SCORES else 0
                                nc.tensor.matmul(
                                    ps[0:m, j * 512:j * 512 + ln],
                                    ktap[hb:hb + 64, moff:moff + m],
                                    qtap[hb:hb + 64, loff:loff + ln],
                                    start=True, stop=True)
                            ex = ep.tile([128, 2048], BF16, tag="ex", name="ex")
                            gk = len(chunks)
                            m0 = MC[chunks[0]][1]
                            if gk > 1:
                                nc.scalar.activation(
                                    ex[0:m0, 0:gk * ln].rearrange(
                                        "p (g n) -> p g n", g=gk),
                                    ps[0:m0, 0:gk * 512].rearrange(
                                        "p (g n) -> p g n", g=gk)[:, :, 0:ln],
                                    AF.Exp, bias=0.0, scale=SCALE)
                            else:
                                nc.scalar.activation(
                                    ex[0:m0, 0:ln], ps[0:m0, 0:ln], AF.Exp,
                                    bias=0.0, scale=SCALE)
                            exs[g] = ex

                        def ctx_mm(g):
                            ex = exs.pop(g)
                            for j, i in enumerate(GROUPS[g]):
                                moff, m = MC[i]
                                nc.tensor.matmul(
                                    pctx[:, 0:ln], v_all[0:m, i, h, :],
                                    ex[0:m, j * ln:j * ln + ln],
                                    start=(i == 0), stop=(i == len(MC) - 1))

                        for g in range(len(GROUPS) + 2):
                            if g < len(GROUPS):
                                scores_exp(g)
                            if g == 2 and pending:
                                pop_finish()
                            if g in (3, 4, 5) and proj_ready:
                                proj_slice(*proj_ready.pop(0))
                            if g >= 2:
                                ctx_mm(g - 2)

                        # park ctx rows + Z row (Z re-based to partition 0);
                        # defer normalize into the next tile's stream
                        zrow = zp.tile([1, 512], F32, tag="zrow", name="zrow")
                        nc.vector.tensor_copy(zrow[0:1, 0:ln], pctx[64:65, 0:ln])
                        ctxu = rzp.tile([64, 512], F32, tag="zc", name="zc")
                        nc.vector.tensor_copy(ctxu[0:64, 0:ln], pctx[0:64, 0:ln])
                        ctxap, hb = CTX_OF_H[h]
                        pending.append((zrow, ctxu, ctxap, hb, loff, ln))

                while pending:
                    pop_finish()
                while proj_ready:
                    proj_slice(*proj_ready.pop(0))
                if DEBUG_DUMP:
                    nc.sync.dma_start(dbg_rk_d[:], rk01[:])
                    nc.sync.dma_start(dbg_kt_d[:], kt_h[0][0:64, :])
                    nc.sync.dma_start(
                        dbg_v_d[:], v_all.rearrange("p a b c -> p (a b c)"))
                    nc.sync.dma_start(dbg_c_d[:], ctx01[:].bitcast(F32))

    nc.finalize()
    return nc


def _rope_tables():
    dim = D // 2
    freqs = 1.0 / 10000 ** (np.arange(0, dim, 2, dtype=np.float64) / dim)
    t = np.arange(GRID, dtype=np.float64)
    f = np.repeat(np.outer(t, freqs), 2, axis=-1)                  # [48, 32]
    fr = np.broadcast_to(f[:, None, :], (GRID, GRID, dim))
    fc = np.broadcast_to(f[None, :, :], (GRID, GRID, dim))
    full = np.concatenate([fr, fc], axis=-1).reshape(GRID * GRID, D)
    cos = np.ones((SEQ, D), np.float64)
    sin = np.zeros((SEQ, D), np.float64)
    cos[TASK:] = np.cos(full)
    sin[TASK:] = np.sin(full)
    return cos.astype(np.float32), sin.astype(np.float32)


def _signed64(tT):
    # [64, S]: signed sine table at the ROTATED (source) rows, so the rope
    # half-multiplies read both operands at equal partition bases.
    return np.ascontiguousarray(np.vstack([tT[32:64], -tT[0:32]]))


def _core_inputs(x, mask, Wqkv, Wproj, bqkv, cos, sin, g, s):
    import ml_dtypes
    bf = ml_dtypes.bfloat16
    xT = x.T  # [768, 2320]
    q0 = SQ * s
    if s == 0:
        perm = None
        xtp = xT
    else:
        perm = np.concatenate([np.arange(SQ, SEQ), np.arange(0, SQ)])
        xtp = np.concatenate([xT[:, SQ:], xT[:, :SQ]], axis=1)
    mkp = mask.astype(np.float32)
    if perm is not None:
        mkp = mkp[perm]
    xtp = xtp * mkp[None, :]   # fold key/value mask into x (V rows -> 0)
    xt = np.ascontiguousarray(
        xtp.reshape(EC, 128, SEQ).transpose(1, 0, 2)).astype(bf)
    r0 = 192 * g

    def wpack(rows):
        w = Wqkv[rows:rows + 192, :].T  # [768, 192]
        return np.ascontiguousarray(
            w.reshape(EC, 128, 192).transpose(1, 0, 2)).astype(bf)

    wq = wpack(r0)
    wk = wpack(768 + r0)
    wv = wpack(1536 + r0)
    wp1 = np.ascontiguousarray(Wproj[:, r0:r0 + 128].T)
    wp2 = np.ascontiguousarray(Wproj[:, r0 + 128:r0 + 192].T)
    b = np.zeros((128, 3), np.float32)
    b[:, 0] = bqkv[768 + r0:768 + r0 + 128]
    b[:, 1] = bqkv[r0:r0 + 128]
    b[0:64, 2] = bqkv[768 + r0 + 128:768 + r0 + 192]
    b[64:128, 2] = bqkv[r0 + 128:r0 + 192]
    cosT, sinT = cos.T, sin.T  # [64, S]
    cq = np.ascontiguousarray(cosT[:, q0:q0 + SQ])
    sq = np.ascontiguousarray(_signed64(sinT)[:, q0:q0 + SQ])
    ckf = cosT
    skf = _signed64(sinT)
    if perm is not None:
        ckf = ckf[:, perm]
        skf = skf[:, perm]
    mk = mask.astype(np.float32)
    if perm is not None:
        mk = mk[perm]
    mk = np.concatenate([mk, np.zeros(19 * 128 - SEQ, np.float32)])
    mk = np.ascontiguousarray(mk.reshape(19, 128).T)
    return {
        "xta": np.ascontiguousarray(xt[:, :, 0:XA]),
        "xtb": np.ascontiguousarray(xt[:, :, XA:XB]),
        "xtc": np.ascontiguousarray(xt[:, :, XB:SEQ]),
        "wq": wq, "wk": wk, "wv": wv,
        "wp1": wp1, "wp2": wp2, "b": b,
        "cq": cq.astype(bf), "sq": sq.astype(bf),
        "ck": np.ascontiguousarray(ckf).astype(bf),
        "sk": np.ascontiguousarray(skf).astype(bf),
        "mk": np.ascontiguousarray(mk),
    }


def _run(x, mask, Wqkv, bqkv, Wproj, bproj, trace=False):
    global _prog
    from concourse.bass_utils import run_bass_kernel_spmd
    if _prog is None:
        _prog = _build()
    x = np.asarray(x, np.float32)
    mask = np.asarray(mask)
    Wqkv = np.asarray(Wqkv, np.float32)
    bqkv = np.asarray(bqkv, np.float32)
    Wproj = np.asarray(Wproj, np.float32)
    bproj = np.asarray(bproj, np.float32)
    cos, sin = _rope_tables()
    in_maps = [
        _core_inputs(x, mask, Wqkv, Wproj, bqkv, cos, sin, core // 2, core % 2)
        for core in range(8)
    ]
    res = run_bass_kernel_spmd(_prog, in_maps, list(range(8)), trace=trace)
    acc = np.zeros((SEQ, E), np.float64)
    for core in range(8):
        s = core % 2
        acc[SQ * s:SQ * (s + 1)] += res.results[core]["pout"].astype(np.float64)
    bias_row = bproj.astype(np.float64) + Wproj.astype(np.float64) @ \
        bqkv[1536:2304].astype(np.float64)
    acc += bias_row
    return acc.astype(np.float32), res


def kernel(x, mask, Wqkv, bqkv, Wproj, bproj):
    out, _ = _run(x, mask, Wqkv, bqkv, Wproj, bproj, trace=False)
    return out


# revision 26
# speedup vs baseline: 1.2037x; 1.2037x over previous
"""Multi-head attention (2D-RoPE, masked softmax) on 8 Trainium2 NeuronCores.

Sharding: 4 head-groups (3 heads each) x 2 query-halves (1160 rows each).
Each core computes full attention for its 3 heads over its 1160 query rows
against all 2320 keys, plus its share of the output projection; the host
sums the 8 partial projections and adds the (folded) biases.

v3 notes:
  - xt ships as 3 separate column-slice tensors (512 / 768 / 1040 cols)
    so every DMA is 128 contiguous multi-KB runs (line rate, not
    descriptor-bound) and phase A starts on slice A at ~11us.
  - No memsets / zero padding.  Score matmuls run as row-tiled concurrent
    pairs (chunk i on PE rows 0-63, chunk i+1 on rows 64-127) against
    kt/qt tiles whose upper 64 partitions duplicate the lower 64
    (per-tile dup copies emitted right after each rope tile).
  - Scores group as quad/pair/quad/... so one PSUM [128,2048] (4 banks) +
    one [128,1024] (2 banks) alternate: exp ACTIVATEs cover 4 chunks at a
    time where possible (63 instead of 90 exp instructions).
  - Head 2's K projection col-tiles with its Q projection (shared moving
    operand, separate PSUM banks - start=True zeroes a whole bank);
    heads 0/1 K and Q project as single M=128 matmuls with one stacked
    [128,n] bias Identity each.
  - Rope runs interleaved into the phase-A job stream (Vector work hides
    under projection matmuls); output projection contracts heads 0+1 as
    one K=128 matmul plus a K=64 matmul for head 2; output stores bf16.
  - finish chain: one [65, ln] PSUM->SBUF copy grabs ctx + Z together,
    reciprocal_approx_fast, normalize multiply reads broadcast PSUM row
    directly; chain + projection slices deferred into the next l-tile's
    instruction stream so the PE never idles at tile boundaries.
  - V-bias and output bias never touch the device:
    out = softmax(..) @ (Vx + bv) @ Wp.T + bp = dev_out + (Wp @ bv + bp).
"""
import sys
if '/opt/trn_rl_repo' not in sys.path:
    sys.path.insert(0, '/opt/trn_rl_repo')
import numpy as np

SEQ, E, NH, D = 2320, 768, 12, 64
GRID, TASK = 48, 16
SQ = SEQ // 2           # query rows per core
HG = 3                  # heads per core
SCALE = D ** -0.5
EC = 6                  # embed chunks of 128
L_TILES = [(0, 512), (512, 392), (904, 256)]
MC = [(i * 128, min(128, SEQ - i * 128)) for i in range(19)]
PT = [(i * 128, min(128, SQ - i * 128)) for i in range(10)]
XA, XB = 512, 1280      # xt slice boundaries: A=[0,512) B=[512,1280) C=[1280,2320)
GROUPS = [(0, 1, 2, 3), (4, 5), (6, 7, 8, 9), (10, 11),
          (12, 13, 14), (15, 16), (17, 18)]
G_TILE = ['Q', 'P', 'Q', 'P', 'Q', 'P', 'Q']  # psum tile per group

ROWTILE_SCORES = True   # concurrent row-tiled score pairs (kt/qt row dup)
COLTILE_KQ2 = True      # head2 K col-tiled with head2 Q
DEBUG_DUMP = False       # dump intermediate tensors as extra outputs

_prog = None


def _build():
    import concourse.mybir as mybir
    import concourse.tile as tile
    from concourse import bacc

    F32, F32R = mybir.dt.float32, mybir.dt.float32r
    BF16 = mybir.dt.bfloat16
    AF = mybir.ActivationFunctionType

    nc = bacc.Bacc('TRN2', target_bir_lowering=False, debug=False, num_devices=8)
    dp = nc.declare_dram_parameter
    xta_d = dp("xta", [128, EC, XA], BF16, isOutput=False)
    xtb_d = dp("xtb", [128, EC, XB - XA], BF16, isOutput=False)
    xtc_d = dp("xtc", [128, EC, SEQ - XB], BF16, isOutput=False)
    wk_d = dp("wk", [128, EC, 192], BF16, isOutput=False)
    wq_d = dp("wq", [128, EC, 192], BF16, isOutput=False)
    wv_d = dp("wv", [128, EC, 192], BF16, isOutput=False)
    wp1_d = dp("wp1", [128, E], F32R, isOutput=False)
    wp2_d = dp("wp2", [64, E], F32R, isOutput=False)
    b_d = dp("b", [128, 3], F32, isOutput=False)
    mk_d = dp("mk", [128, 19], F32, isOutput=False)
    ck_d = dp("ck", [64, SEQ], BF16, isOutput=False)
    sk_d = dp("sk", [64, SEQ], BF16, isOutput=False)
    cq_d = dp("cq", [64, SQ], BF16, isOutput=False)
    sq_d = dp("sq", [64, SQ], BF16, isOutput=False)
    out_d = dp("pout", [SQ, E], BF16, isOutput=True)
    if DEBUG_DUMP:
        dbg_rk_d = dp("dbg_rk01", [128, SEQ], BF16, isOutput=True)
        dbg_kt_d = dp("dbg_kt0", [64, SEQ], BF16, isOutput=True)
        dbg_v_d = dp("dbg_vall", [128, 19 * HG * 65], BF16, isOutput=True)
        dbg_c_d = dp("dbg_ctx", [128, SQ], F32, isOutput=True)

    with tile.TileContext(nc) as tc:
        with (
            tc.tile_pool(name="long", bufs=1) as lp,
            tc.tile_pool(name="zp", bufs=2) as zp,
        ):
            xta = lp.tile([128, EC, XA], BF16, tag="xta")
            xtb = lp.tile([128, EC, XB - XA], BF16, tag="xtb")
            xtc = lp.tile([128, EC, SEQ - XB], BF16, tag="xtc")
            wk_sb = lp.tile([128, EC, 192], BF16, tag="wk")
            wq_sb = lp.tile([128, EC, 192], BF16, tag="wq")
            wv_sb = lp.tile([128, EC, 192], BF16, tag="wv")
            wp1_sb = lp.tile([128, E], F32R, tag="wp1")
            wp2_sb = lp.tile([64, E], F32R, tag="wp2")
            b_sb = lp.tile([128, 3], F32, tag="b")
            mk_sb = lp.tile([128, 19], F32, tag="mk")
            ck_sb = lp.tile([128, SEQ], BF16, tag="ck")
            sk_sb = lp.tile([128, SEQ], BF16, tag="sk")
            cq_sb = lp.tile([128, SQ], BF16, tag="cq")
            sq_sb = lp.tile([128, SQ], BF16, tag="sq")
            rk01 = lp.tile([128, SEQ], BF16, tag="rk01")
            rq01 = lp.tile([128, SQ], BF16, tag="rq01")
            raw2 = lp.tile([128, SEQ], BF16, tag="raw2")
            kt_h = [lp.tile([128, SEQ], BF16, tag=f"kt{h}", name=f"kt{h}")
                    for h in range(HG)]
            qt_h = [lp.tile([128, SQ], BF16, tag=f"qt{h}", name=f"qt{h}")
                    for h in range(HG)]
            v_all = lp.tile([128, 19, HG, 65], BF16, tag="v_all")
            ctx01 = lp.tile([128, SQ], F32R, tag="ctx01")
            ctx2 = lp.tile([64, SQ], F32R, tag="ctx2")
            ones64 = lp.tile([1, 64], F32R, tag="ones64")

            nc.gpsimd.memset(ones64[:].bitcast(F32), 1.0)

            # ---- input DMAs: few, large, spread across issue paths ----
            nc.gpsimd.dma_start(wk_sb[:], wk_d[:])
            nc.gpsimd.dma_start(b_sb[:], b_d[:])
            nc.gpsimd.dma_start(wq_sb[:], wq_d[:])
            nc.gpsimd.dma_start(wv_sb[:], wv_d[:])
            nc.gpsimd.dma_start(mk_sb[:], mk_d[:])
            nc.sync.dma_start(xta[:], xta_d[:])
            nc.sync.dma_start(xtb[:], xtb_d[:])
            nc.sync.dma_start(xtc[:], xtc_d[:])
            nc.sync.dma_start(wp1_sb[:], wp1_d[:])
            nc.sync.dma_start(wp2_sb[:], wp2_d[:])
            nc.gpsimd.dma_start(ck_sb[0:64, :], ck_d[:])
            nc.gpsimd.dma_start(sk_sb[0:64, :], sk_d[:])
            nc.gpsimd.dma_start(cq_sb[0:64, :], cq_d[:])
            nc.gpsimd.dma_start(sq_sb[0:64, :], sq_d[:])
            nc.vector.tensor_copy(ck_sb[64:128, :], ck_sb[0:64, :])
            nc.vector.tensor_copy(sk_sb[64:128, :], sk_sb[0:64, :])
            nc.vector.tensor_copy(cq_sb[64:128, :], cq_sb[0:64, :])
            nc.vector.tensor_copy(sq_sb[64:128, :], sq_sb[0:64, :])
            # softmax-denominator ones column for all chunks at once
            nc.vector.tensor_copy(
                v_all[:, :, :, 64:65],
                mk_sb[:, :].to_broadcast([128, 19, HG, 1]))

            def xt_of(off, n):
                if off + n <= XA:
                    return xta, off
                if off + n <= XB:
                    return xtb, off - XA
                return xtc, off - XB

            # ---- phase A: QKV projections (+ interleaved rope) ----
            with tc.tile_pool(name="pk", bufs=3, space="PSUM") as pkp, \
                 tc.tile_pool(name="pv", bufs=2, space="PSUM") as pvp:

                def v_tile(i):
                    # mask folded into xt on the host (zeroed columns), so
                    # the PSUM->SBUF move runs on ScalarE, not Vector
                    off, m = MC[i]
                    xt, lo = xt_of(off, m)
                    pv = pvp.tile([128, 192], F32, tag="pv", name="pv")
                    for c in range(EC):
                        nc.tensor.matmul(
                            pv[0:m, :], xt[:, c, lo:lo + m], wv_sb[:, c, :],
                            start=(c == 0), stop=(c == EC - 1))
                    nc.scalar.activation(
                        v_all[0:m, i, :, 0:64],
                        pv[0:m, 0:192].rearrange("p (h d) -> p h d", h=HG),
                        AF.Identity, bias=0.0, scale=1.0)

                def mm128(w_sb, bcol, rawt, off, n):
                    xt, lo = xt_of(off, n)
                    ps = pkp.tile([128, 512], F32, tag="pk", name="pk")
                    for c in range(EC):
                        nc.tensor.matmul(
                            ps[0:128, 0:n], w_sb[:, c, 0:128], xt[:, c, lo:lo + n],
                            start=(c == 0), stop=(c == EC - 1))
                    nc.scalar.activation(
                        rawt[0:128, off:off + n], ps[0:128, 0:n], AF.Identity,
                        bias=b_sb[:, bcol:bcol + 1], scale=1.0)

                def kq2(off, n):
                    # head2 K and Q over the query range; when col-tiled the
                    # two chains share the moving xt operand but use separate
                    # PSUM banks (start=True zeroes a whole bank).
                    xt, lo = xt_of(off, n)
                    psa = pkp.tile([128, 512], F32, tag="pk", name="psa")
                    psb = pkp.tile([128, 512], F32, tag="pk", name="psb")
                    qrow = 64 if COLTILE_KQ2 else 0
                    for c in range(EC):
                        nc.tensor.matmul(
                            psa[0:64, 0:n], wk_sb[:, c, 128:192], xt[:, c, lo:lo + n],
                            start=(c == 0), stop=(c == EC - 1))
                        nc.tensor.matmul(
                            psb[qrow:qrow + 64, 0:n], wq_sb[:, c, 128:192],
                            xt[:, c, lo:lo + n],
                            start=(c == 0), stop=(c == EC - 1))
                    nc.scalar.activation(
                        raw2[0:64, off:off + n], psa[0:64, 0:n], AF.Identity,
                        bias=b_sb[0:64, 2:3], scale=1.0)
                    nc.scalar.activation(
                        raw2[64:128, off:off + n], psb[qrow:qrow + 64, 0:n],
                        AF.Identity, bias=b_sb[64:128, 2:3], scale=1.0)

                def k2a(off, n):
                    xt, lo = xt_of(off, n)
                    ps = pkp.tile([128, 512], F32, tag="pk", name="pk")
                    for c in range(EC):
                        nc.tensor.matmul(
                            ps[0:64, 0:n], wk_sb[:, c, 128:192], xt[:, c, lo:lo + n],
                            start=(c == 0), stop=(c == EC - 1))
                    nc.scalar.activation(
                        raw2[0:64, off:off + n], ps[0:64, 0:n], AF.Identity,
                        bias=b_sb[0:64, 2:3], scale=1.0)

                # ---- rope (SBUF-only; interleaves with projection PE work)
                def rope1(rawt, hb, cos_sb, sin_sb, off, n, outt):
                    t1 = zp.tile([64, 512], BF16, tag="rt1", name="rt1", bufs=3)
                    t2 = zp.tile([64, 512], BF16, tag="rt2", name="rt2", bufs=3)
                    nc.vector.tensor_mul(
                        t1[0:64, 0:n], rawt[hb:hb + 64, off:off + n],
                        cos_sb[hb:hb + 64, off:off + n])
                    for bi in range(2):
                        src = hb + bi * 32 + (32 if bi % 2 == 0 else -32)
                        nc.vector.tensor_mul(
                            t2[bi * 32:(bi + 1) * 32, 0:n],
                            rawt[src:src + 32, off:off + n],
                            sin_sb[src:src + 32, off:off + n])
                    nc.vector.tensor_add(
                        outt[0:64, off:off + n], t1[0:64, 0:n], t2[0:64, 0:n])
                    if ROWTILE_SCORES:
                        # duplicate rows for the row-tiled score pairs
                        nc.vector.tensor_copy(
                            outt[64:128, off:off + n], outt[0:64, off:off + n])

                def ropek01(off, n):
                    rope1(rk01, 0, ck_sb, sk_sb, off, n, kt_h[0])
                    rope1(rk01, 64, ck_sb, sk_sb, off, n, kt_h[1])

                def ropeq01(off, n):
                    rope1(rq01, 0, cq_sb, sq_sb, off, n, qt_h[0])
                    rope1(rq01, 64, cq_sb, sq_sb, off, n, qt_h[1])

                jobs = [
                    ("k01", wk_d, (0, 512)), ("rk01", (0, 512)),
                    ("kq2", (0, 512)), ("rk2", (0, 512)), ("rq2", (0, 512)),
                    ("q01", wq_d, (0, 512)), ("rq01", (0, 512)),
                    ("v", 0), ("v", 1), ("v", 2), ("v", 3),
                    ("k01", wk_d, (512, 512)), ("rk01", (512, 512)),
                    ("kq2", (512, 392)),
                    ("q01", wq_d, (512, 392)), ("rq01", (512, 392)),
                    ("v", 4), ("v", 5),
                    ("k01", wk_d, (1024, 256)),
                    ("kq2", (904, 256)), ("rk2", (512, 512)),
                    ("rq2", (512, 392)), ("rq2", (904, 256)),
                    ("q01", wq_d, (904, 256)), ("rq01", (904, 256)),
                    ("k2a", (1160, 120)),
                    ("v", 6), ("v", 7), ("v", 8), ("v", 9),
                    ("k01", wk_d, (1280, 512)), ("rk01", (1024, 512)),
                    ("k2a", (1280, 512)), ("rk2", (1024, 512)),
                    ("v", 10), ("v", 11), ("v", 12),
                    ("k01", wk_d, (1792, 512)), ("rk01", (1536, 512)),
                    ("k2a", (1792, 512)), ("rk2", (1536, 512)),
                    ("v", 13), ("v", 14), ("v", 15),
                    ("k01", wk_d, (2304, 16)), ("rk01", (2048, 272)),
                    ("k2a", (2304, 16)), ("rk2", (2048, 272)),
                    ("v", 16), ("v", 17), ("v", 18),
                ]
                for job in jobs:
                    kind = job[0]
                    if kind == "v":
                        v_tile(job[1])
                    elif kind == "k01":
                        mm128(wk_sb, 0, rk01, *job[2])
                    elif kind == "q01":
                        mm128(wq_sb, 1, rq01, *job[2])
                    elif kind == "kq2":
                        kq2(*job[1])
                    elif kind == "k2a":
                        k2a(*job[1])
                    elif kind == "rk01":
                        ropek01(*job[1])
                    elif kind == "rq01":
                        ropeq01(*job[1])
                    elif kind == "rk2":
                        rope1(raw2, 0, ck_sb, sk_sb, *job[1], kt_h[2])
                    elif kind == "rq2":
                        rope1(raw2, 64, cq_sb, sq_sb, *job[1], qt_h[2])

            # ---- attention ----
            with tc.tile_pool(name="ep", bufs=3) as ep, \
                 tc.tile_pool(name="op", bufs=2) as op, \
                 tc.tile_pool(name="rzp", bufs=2) as rzp, \
                 tc.tile_pool(name="psQ", bufs=1, space="PSUM") as psQ, \
                 tc.tile_pool(name="psP", bufs=1, space="PSUM") as psP, \
                 tc.tile_pool(name="pc3", bufs=1, space="PSUM") as pc3, \
                 tc.tile_pool(name="pp3", bufs=1, space="PSUM") as pp3:
                PROJ_OF_LT = {0: PT[0:4], 1: PT[4:7], 2: PT[7:10]}
                CTX_OF_H = [(ctx01, 0), (ctx01, 64), (ctx2, 0)]

                def proj_slice(toff, tm):
                    outsb = op.tile([128, E], BF16, tag="outsb", name="outsb")
                    for half in range(2):
                        hs = half * 384
                        pp = pp3.tile([128, 512], F32, tag="pp", name="pp")
                        nc.tensor.matmul(
                            pp[0:tm, 0:384], ctx01[0:128, toff:toff + tm],
                            wp1_sb[0:128, hs:hs + 384], start=True, stop=False)
                        nc.tensor.matmul(
                            pp[0:tm, 0:384], ctx2[0:64, toff:toff + tm],
                            wp2_sb[0:64, hs:hs + 384], start=False, stop=True)
                        nc.vector.tensor_copy(outsb[0:tm, hs:hs + 384], pp[0:tm, 0:384])
                    nc.sync.dma_start(out_d[toff:toff + tm, :], outsb[0:tm, :])

                pending = []       # deferred normalize jobs
                proj_ready = []    # proj slices whose ctx inputs are complete

                def finish_tile(z):
                    # NOTE: custom DVE ops (reciprocal_approx_*) must read
                    # and write at the SAME partition base on hardware - the
                    # Z row is parked at base 0 (zrow) before the recip.
                    zrow, ctxu, ctxap, hb, loff2, ln2 = z
                    rzf = zp.tile([1, 512], F32, tag="rzf", name="rzf")
                    nc.vector.reciprocal_approx_fast(rzf[0:1, 0:ln2], zrow[0:1, 0:ln2])
                    rzr = zp.tile([1, 512], F32R, tag="rzr", name="rzr")
                    nc.vector.tensor_copy(rzr[0:1, 0:ln2], rzf[0:1, 0:ln2])
                    przb = pp3.tile([128, 512], F32, tag="pp", name="przb")
                    nc.tensor.matmul(
                        przb[0:64, 0:ln2], ones64[:], rzr[0:1, 0:ln2],
                        start=True, stop=True)
                    nc.vector.tensor_mul(
                        ctxap[hb:hb + 64, loff2:loff2 + ln2],
                        ctxu[0:64, 0:ln2], przb[0:64, 0:ln2])

                def pop_finish():
                    z = pending.pop(0)
                    finish_tile(z)
                    if z[2] is ctx2:
                        proj_ready.extend(PROJ_OF_LT[L_TILES.index((z[4], z[5]))])

                for lt_i, (loff, ln) in enumerate(L_TILES):
                    for h in range(HG):
                        ktap, qtap = kt_h[h], qt_h[h]
                        pctx = pc3.tile([65, 512], F32, tag="pctx")
                        exs = {}

                        def scores_exp(g):
                            chunks = GROUPS[g]
                            if G_TILE[g] == 'Q':
                                ps = psQ.tile([128, 2048], F32, tag="psq", name="psq")
                            elif G_TILE[g] == 'P':
                                ps = psP.tile([128, 1024], F32, tag="psp", name="psp")
                            else:
                                ps = pp3.tile([128, 512], F32, tag="pp", name="pss")
                            for j, i in enumerate(chunks):
                                moff, m = MC[i]
                                hb = 64 * (j % 2) if ROWTILE_SCORES else 0
                                nc.tensor.matmul(
                                    ps[0:m, j * 512:j * 512 + ln],
                                    ktap[hb:hb + 64, moff:moff + m],
                                    qtap[hb:hb + 64, loff:loff + ln],
                                    start=True, stop=True)
                            ex = ep.tile([128, 2048], BF16, tag="ex", name="ex")
                            gk = len(chunks)
                            m0 = MC[chunks[0]][1]
                            if gk > 1:
                                nc.scalar.activation(
                                    ex[0:m0, 0:gk * ln].rearrange(
                                        "p (g n) -> p g n", g=gk),
                                    ps[0:m0, 0:gk * 512].rearrange(
                                        "p (g n) -> p g n", g=gk)[:, :, 0:ln],
                                    AF.Exp, bias=0.0, scale=SCALE)
                            else:
                                nc.scalar.activation(
                                    ex[0:m0, 0:ln], ps[0:m0, 0:ln], AF.Exp,
                                    bias=0.0, scale=SCALE)
                            exs[g] = ex

                        def ctx_mm(g):
                            ex = exs.pop(g)
                            for j, i in enumerate(GROUPS[g]):
                                moff, m = MC[i]
                                nc.tensor.matmul(
                                    pctx[:, 0:ln], v_all[0:m, i, h, :],
                                    ex[0:m, j * ln:j * ln + ln],
                                    start=(i == 0), stop=(i == len(MC) - 1))

                        for g in range(len(GROUPS) + 2):
                            if g < len(GROUPS):
                                scores_exp(g)
                            if g == 2 and pending:
                                pop_finish()
                            if g in (3, 4, 5) and proj_ready:
                                proj_slice(*proj_ready.pop(0))
                            if g >= 2:
                                ctx_mm(g - 2)

                        # park ctx rows + Z row (Z re-based to partition 0);
                        # defer normalize into the next tile's stream
                        zrow = zp.tile([1, 512], F32, tag="zrow", name="zrow")
                        nc.vector.tensor_copy(zrow[0:1, 0:ln], pctx[64:65, 0:ln])
                        ctxu = rzp.tile([64, 512], F32, tag="zc", name="zc")
                        nc.vector.tensor_copy(ctxu[0:64, 0:ln], pctx[0:64, 0:ln])
                        ctxap, hb = CTX_OF_H[h]
                        pending.append((zrow, ctxu, ctxap, hb, loff, ln))

                while pending:
                    pop_finish()
                while proj_ready:
                    proj_slice(*proj_ready.pop(0))
                if DEBUG_DUMP:
                    nc.sync.dma_start(dbg_rk_d[:], rk01[:])
                    nc.sync.dma_start(dbg_kt_d[:], kt_h[0][0:64, :])
                    nc.sync.dma_start(
                        dbg_v_d[:], v_all.rearrange("p a b c -> p (a b c)"))
                    nc.sync.dma_start(dbg_c_d[:], ctx01[:].bitcast(F32))

    nc.finalize()
    return nc


def _rope_tables():
    dim = D // 2
    freqs = 1.0 / 10000 ** (np.arange(0, dim, 2, dtype=np.float64) / dim)
    t = np.arange(GRID, dtype=np.float64)
    f = np.repeat(np.outer(t, freqs), 2, axis=-1)                  # [48, 32]
    fr = np.broadcast_to(f[:, None, :], (GRID, GRID, dim))
    fc = np.broadcast_to(f[None, :, :], (GRID, GRID, dim))
    full = np.concatenate([fr, fc], axis=-1).reshape(GRID * GRID, D)
    cos = np.ones((SEQ, D), np.float64)
    sin = np.zeros((SEQ, D), np.float64)
    cos[TASK:] = np.cos(full)
    sin[TASK:] = np.sin(full)
    return cos.astype(np.float32), sin.astype(np.float32)


def _signed64(tT):
    # [64, S]: signed sine table at the ROTATED (source) rows, so the rope
    # half-multiplies read both operands at equal partition bases.
    return np.ascontiguousarray(np.vstack([tT[32:64], -tT[0:32]]))


def _core_inputs(x, mask, Wqkv, Wproj, bqkv, cos, sin, g, s):
    import ml_dtypes
    bf = ml_dtypes.bfloat16
    xT = x.T  # [768, 2320]
    q0 = SQ * s
    if s == 0:
        perm = None
        xtp = xT
    else:
        perm = np.concatenate([np.arange(SQ, SEQ), np.arange(0, SQ)])
        xtp = np.concatenate([xT[:, SQ:], xT[:, :SQ]], axis=1)
    mkp = mask.astype(np.float32)
    if perm is not None:
        mkp = mkp[perm]
    xtp = xtp * mkp[None, :]   # fold key/value mask into x (V rows -> 0)
    xt = np.ascontiguousarray(
        xtp.reshape(EC, 128, SEQ).transpose(1, 0, 2)).astype(bf)
    r0 = 192 * g

    def wpack(rows):
        w = Wqkv[rows:rows + 192, :].T  # [768, 192]
        return np.ascontiguousarray(
            w.reshape(EC, 128, 192).transpose(1, 0, 2)).astype(bf)

    wq = wpack(r0)
    wk = wpack(768 + r0)
    wv = wpack(1536 + r0)
    wp1 = np.ascontiguousarray(Wproj[:, r0:r0 + 128].T)
    wp2 = np.ascontiguousarray(Wproj[:, r0 + 128:r0 + 192].T)
    b = np.zeros((128, 3), np.float32)
    b[:, 0] = bqkv[768 + r0:768 + r0 + 128]
    b[:, 1] = bqkv[r0:r0 + 128]
    b[0:64, 2] = bqkv[768 + r0 + 128:768 + r0 + 192]
    b[64:128, 2] = bqkv[r0 + 128:r0 + 192]
    cosT, sinT = cos.T, sin.T  # [64, S]
    cq = np.ascontiguousarray(cosT[:, q0:q0 + SQ])
    sq = np.ascontiguousarray(_signed64(sinT)[:, q0:q0 + SQ])
    ckf = cosT
    skf = _signed64(sinT)
    if perm is not None:
        ckf = ckf[:, perm]
        skf = skf[:, perm]
    mk = mask.astype(np.float32)
    if perm is not None:
        mk = mk[perm]
    mk = np.concatenate([mk, np.zeros(19 * 128 - SEQ, np.float32)])
    mk = np.ascontiguousarray(mk.reshape(19, 128).T)
    return {
        "xta": np.ascontiguousarray(xt[:, :, 0:XA]),
        "xtb": np.ascontiguousarray(xt[:, :, XA:XB]),
        "xtc": np.ascontiguousarray(xt[:, :, XB:SEQ]),
        "wq": wq, "wk": wk, "wv": wv,
        "wp1": wp1, "wp2": wp2, "b": b,
        "cq": cq.astype(bf), "sq": sq.astype(bf),
        "ck": np.ascontiguousarray(ckf).astype(bf),
        "sk": np.ascontiguousarray(skf).astype(bf),
        "mk": np.ascontiguousarray(mk),
    }


def _run(x, mask, Wqkv, bqkv, Wproj, bproj, trace=False):
    global _prog
    from concourse.bass_utils import run_bass_kernel_spmd
    if _prog is None:
        _prog = _build()
    x = np.asarray(x, np.float32)
    mask = np.asarray(mask)
    Wqkv = np.asarray(Wqkv, np.float32)
    bqkv = np.asarray(bqkv, np.float32)
    Wproj = np.asarray(Wproj, np.float32)
    bproj = np.asarray(bproj, np.float32)
    cos, sin = _rope_tables()
    in_maps = [
        _core_inputs(x, mask, Wqkv, Wproj, bqkv, cos, sin, core // 2, core % 2)
        for core in range(8)
    ]
    res = run_bass_kernel_spmd(_prog, in_maps, list(range(8)), trace=trace)
    acc = np.zeros((SEQ, E), np.float64)
    for core in range(8):
        s = core % 2
        acc[SQ * s:SQ * (s + 1)] += res.results[core]["pout"].astype(np.float64)
    bias_row = bproj.astype(np.float64) + Wproj.astype(np.float64) @ \
        bqkv[1536:2304].astype(np.float64)
    acc += bias_row
    return acc.astype(np.float32), res


def kernel(x, mask, Wqkv, bqkv, Wproj, bproj):
    out, _ = _run(x, mask, Wqkv, bqkv, Wproj, bproj, trace=False)
    return out


# revision 31
# speedup vs baseline: 1.2739x; 1.0583x over previous
"""Multi-head attention (2D-RoPE, masked softmax) on 8 Trainium2 NeuronCores.

Sharding: 4 head-groups (3 heads each) x 2 query-halves (1160 rows each).
Each core computes full attention for its 3 heads over its 1160 query rows
against all 2320 keys, plus its share of the output projection; the host
sums the 8 partial projections and adds the (folded) biases.

v3 notes:
  - xt ships as 3 separate column-slice tensors (512 / 768 / 1040 cols)
    so every DMA is 128 contiguous multi-KB runs (line rate, not
    descriptor-bound) and phase A starts on slice A at ~11us.
  - No memsets / zero padding.  Score matmuls run as row-tiled concurrent
    pairs (chunk i on PE rows 0-63, chunk i+1 on rows 64-127) against
    kt/qt tiles whose upper 64 partitions duplicate the lower 64
    (per-tile dup copies emitted right after each rope tile).
  - Scores group as quad/pair/quad/... so one PSUM [128,2048] (4 banks) +
    one [128,1024] (2 banks) alternate: exp ACTIVATEs cover 4 chunks at a
    time where possible (63 instead of 90 exp instructions).
  - Head 2's K projection col-tiles with its Q projection (shared moving
    operand, separate PSUM banks - start=True zeroes a whole bank);
    heads 0/1 K and Q project as single M=128 matmuls with one stacked
    [128,n] bias Identity each.
  - Rope runs interleaved into the phase-A job stream (Vector work hides
    under projection matmuls); output projection contracts heads 0+1 as
    one K=128 matmul plus a K=64 matmul for head 2; output stores bf16.
  - finish chain: one [65, ln] PSUM->SBUF copy grabs ctx + Z together,
    reciprocal_approx_fast, normalize multiply reads broadcast PSUM row
    directly; chain + projection slices deferred into the next l-tile's
    instruction stream so the PE never idles at tile boundaries.
  - V-bias and output bias never touch the device:
    out = softmax(..) @ (Vx + bv) @ Wp.T + bp = dev_out + (Wp @ bv + bp).
"""
import sys
if '/opt/trn_rl_repo' not in sys.path:
    sys.path.insert(0, '/opt/trn_rl_repo')
import numpy as np

SEQ, E, NH, D = 2320, 768, 12, 64
GRID, TASK = 48, 16
SQ = SEQ // 2           # query rows per core
HG = 3                  # heads per core
SCALE = D ** -0.5
EC = 6                  # embed chunks of 128
L_TILES = [(0, 512), (512, 392), (904, 256)]
MC = [(i * 128, min(128, SEQ - i * 128)) for i in range(19)]
PT = [(i * 128, min(128, SQ - i * 128)) for i in range(10)]
XA, XB = 512, 1280      # xt slice boundaries: A=[0,512) B=[512,1280) C=[1280,2320)
# two chunk-group layouts, alternated per head so consecutive heads never
# allocate the same PSUM pool back-to-back (no cross-head pipeline bubble)
GROUPS_A = [(0, 1, 2, 3), (4, 5), (6, 7, 8, 9), (10, 11),
            (12, 13, 14), (15, 16), (17, 18)]
G_TILE_A = ['Q', 'P', 'Q', 'P', 'Q', 'P', 'Q']
GROUPS_B = [(0, 1), (2, 3, 4, 5), (6, 7), (8, 9, 10, 11),
            (12, 13), (14, 15, 16), (17, 18)]
G_TILE_B = ['P', 'Q', 'P', 'Q', 'P', 'Q', 'P']

ROWTILE_SCORES = True   # concurrent row-tiled score pairs (kt/qt row dup)
COLTILE_KQ2 = True      # head2 K col-tiled with head2 Q
DEBUG_DUMP = False       # dump intermediate tensors as extra outputs

_prog = None


def _build():
    import concourse.mybir as mybir
    import concourse.tile as tile
    from concourse import bacc

    F32, F32R = mybir.dt.float32, mybir.dt.float32r
    BF16 = mybir.dt.bfloat16
    AF = mybir.ActivationFunctionType

    nc = bacc.Bacc('TRN2', target_bir_lowering=False, debug=False, num_devices=8)
    dp = nc.declare_dram_parameter
    xta_d = dp("xta", [128, EC, XA], BF16, isOutput=False)
    xtb_d = dp("xtb", [128, EC, XB - XA], BF16, isOutput=False)
    xtc_d = dp("xtc", [128, EC, SEQ - XB], BF16, isOutput=False)
    wk_d = dp("wk", [128, EC, 192], BF16, isOutput=False)
    wq_d = dp("wq", [128, EC, 192], BF16, isOutput=False)
    wv_d = dp("wv", [128, EC, 192], BF16, isOutput=False)
    wp1_d = dp("wp1", [128, E], F32R, isOutput=False)
    wp2_d = dp("wp2", [64, E], F32R, isOutput=False)
    b_d = dp("b", [128, 3], F32, isOutput=False)
    mk_d = dp("mk", [128, 19], F32, isOutput=False)
    ck_d = dp("ck", [64, SEQ], BF16, isOutput=False)
    sk_d = dp("sk", [64, SEQ], BF16, isOutput=False)
    cq_d = dp("cq", [64, SQ], BF16, isOutput=False)
    sq_d = dp("sq", [64, SQ], BF16, isOutput=False)
    out_d = dp("pout", [SQ, E], BF16, isOutput=True)
    if DEBUG_DUMP:
        dbg_rk_d = dp("dbg_rk01", [128, SEQ], BF16, isOutput=True)
        dbg_kt_d = dp("dbg_kt0", [64, SEQ], BF16, isOutput=True)
        dbg_v_d = dp("dbg_vall", [128, 19 * HG * 65], BF16, isOutput=True)
        dbg_c_d = dp("dbg_ctx", [128, SQ], F32, isOutput=True)

    with tile.TileContext(nc) as tc:
        with (
            tc.tile_pool(name="long", bufs=1) as lp,
            tc.tile_pool(name="zp", bufs=2) as zp,
        ):
            xta = lp.tile([128, EC, XA], BF16, tag="xta")
            xtb = lp.tile([128, EC, XB - XA], BF16, tag="xtb")
            xtc = lp.tile([128, EC, SEQ - XB], BF16, tag="xtc")
            wk_sb = lp.tile([128, EC, 192], BF16, tag="wk")
            wq_sb = lp.tile([128, EC, 192], BF16, tag="wq")
            wv_sb = lp.tile([128, EC, 192], BF16, tag="wv")
            wp1_sb = lp.tile([128, E], F32R, tag="wp1")
            wp2_sb = lp.tile([64, E], F32R, tag="wp2")
            b_sb = lp.tile([128, 3], F32, tag="b")
            mk_sb = lp.tile([128, 19], F32, tag="mk")
            ck_sb = lp.tile([128, SEQ], BF16, tag="ck")
            sk_sb = lp.tile([128, SEQ], BF16, tag="sk")
            cq_sb = lp.tile([128, SQ], BF16, tag="cq")
            sq_sb = lp.tile([128, SQ], BF16, tag="sq")
            rk01 = lp.tile([128, SEQ], BF16, tag="rk01")
            rq01 = lp.tile([128, SQ], BF16, tag="rq01")
            raw2 = lp.tile([128, SEQ], BF16, tag="raw2")
            kt_h = [lp.tile([128, SEQ], BF16, tag=f"kt{h}", name=f"kt{h}")
                    for h in range(HG)]
            qt_h = [lp.tile([128, SQ], BF16, tag=f"qt{h}", name=f"qt{h}")
                    for h in range(HG)]
            v_all = lp.tile([128, 19, HG, 65], BF16, tag="v_all")
            ctx01 = lp.tile([128, SQ], F32R, tag="ctx01")
            ctx2 = lp.tile([64, SQ], F32R, tag="ctx2")
            ones64 = lp.tile([1, 64], F32R, tag="ones64")

            nc.gpsimd.memset(ones64[:].bitcast(F32), 1.0)

            # ---- input DMAs: few, large, spread across issue paths ----
            nc.gpsimd.dma_start(wk_sb[:], wk_d[:])
            nc.gpsimd.dma_start(b_sb[:], b_d[:])
            nc.gpsimd.dma_start(wq_sb[:], wq_d[:])
            nc.gpsimd.dma_start(wv_sb[:], wv_d[:])
            nc.gpsimd.dma_start(mk_sb[:], mk_d[:])
            nc.sync.dma_start(xta[:], xta_d[:])
            nc.sync.dma_start(xtb[:], xtb_d[:])
            nc.sync.dma_start(xtc[:], xtc_d[:])
            nc.sync.dma_start(wp1_sb[:], wp1_d[:])
            nc.sync.dma_start(wp2_sb[:], wp2_d[:])
            nc.gpsimd.dma_start(ck_sb[0:64, :], ck_d[:])
            nc.gpsimd.dma_start(sk_sb[0:64, :], sk_d[:])
            nc.gpsimd.dma_start(cq_sb[0:64, :], cq_d[:])
            nc.gpsimd.dma_start(sq_sb[0:64, :], sq_d[:])
            nc.vector.tensor_copy(ck_sb[64:128, :], ck_sb[0:64, :])
            nc.vector.tensor_copy(sk_sb[64:128, :], sk_sb[0:64, :])
            nc.vector.tensor_copy(cq_sb[64:128, :], cq_sb[0:64, :])
            nc.vector.tensor_copy(sq_sb[64:128, :], sq_sb[0:64, :])
            # softmax-denominator ones column for all chunks at once
            nc.vector.tensor_copy(
                v_all[:, :, :, 64:65],
                mk_sb[:, :].to_broadcast([128, 19, HG, 1]))

            def xt_of(off, n):
                if off + n <= XA:
                    return xta, off
                if off + n <= XB:
                    return xtb, off - XA
                return xtc, off - XB

            # ---- phase A: QKV projections (+ interleaved rope) ----
            with tc.tile_pool(name="pk", bufs=3, space="PSUM") as pkp, \
                 tc.tile_pool(name="pv", bufs=2, space="PSUM") as pvp:

                def v_tile(i):
                    # mask folded into xt on the host (zeroed columns), so
                    # the PSUM->SBUF move runs on ScalarE, not Vector
                    off, m = MC[i]
                    xt, lo = xt_of(off, m)
                    pv = pvp.tile([128, 192], F32, tag="pv", name="pv")
                    for c in range(EC):
                        nc.tensor.matmul(
                            pv[0:m, :], xt[:, c, lo:lo + m], wv_sb[:, c, :],
                            start=(c == 0), stop=(c == EC - 1))
                    nc.scalar.activation(
                        v_all[0:m, i, :, 0:64],
                        pv[0:m, 0:192].rearrange("p (h d) -> p h d", h=HG),
                        AF.Identity, bias=0.0, scale=1.0)

                def mm128(w_sb, bcol, rawt, off, n):
                    xt, lo = xt_of(off, n)
                    ps = pkp.tile([128, 512], F32, tag="pk", name="pk")
                    for c in range(EC):
                        nc.tensor.matmul(
                            ps[0:128, 0:n], w_sb[:, c, 0:128], xt[:, c, lo:lo + n],
                            start=(c == 0), stop=(c == EC - 1))
                    nc.scalar.activation(
                        rawt[0:128, off:off + n], ps[0:128, 0:n], AF.Identity,
                        bias=b_sb[:, bcol:bcol + 1], scale=1.0)

                def kq2(off, n):
                    # head2 K and Q over the query range; when col-tiled the
                    # two chains share the moving xt operand but use separate
                    # PSUM banks (start=True zeroes a whole bank).
                    xt, lo = xt_of(off, n)
                    psa = pkp.tile([128, 512], F32, tag="pk", name="psa")
                    psb = pkp.tile([128, 512], F32, tag="pk", name="psb")
                    qrow = 64 if COLTILE_KQ2 else 0
                    for c in range(EC):
                        nc.tensor.matmul(
                            psa[0:64, 0:n], wk_sb[:, c, 128:192], xt[:, c, lo:lo + n],
                            start=(c == 0), stop=(c == EC - 1))
                        nc.tensor.matmul(
                            psb[qrow:qrow + 64, 0:n], wq_sb[:, c, 128:192],
                            xt[:, c, lo:lo + n],
                            start=(c == 0), stop=(c == EC - 1))
                    nc.scalar.activation(
                        raw2[0:64, off:off + n], psa[0:64, 0:n], AF.Identity,
                        bias=b_sb[0:64, 2:3], scale=1.0)
                    nc.scalar.activation(
                        raw2[64:128, off:off + n], psb[qrow:qrow + 64, 0:n],
                        AF.Identity, bias=b_sb[64:128, 2:3], scale=1.0)

                def k2a(off, n):
                    xt, lo = xt_of(off, n)
                    ps = pkp.tile([128, 512], F32, tag="pk", name="pk")
                    for c in range(EC):
                        nc.tensor.matmul(
                            ps[0:64, 0:n], wk_sb[:, c, 128:192], xt[:, c, lo:lo + n],
                            start=(c == 0), stop=(c == EC - 1))
                    nc.scalar.activation(
                        raw2[0:64, off:off + n], ps[0:64, 0:n], AF.Identity,
                        bias=b_sb[0:64, 2:3], scale=1.0)

                # ---- rope (SBUF-only; interleaves with projection PE work)
                def rope_mul(rawt, p0, np_, cos_sb, sin_sb, off, n):
                    # t1/t2 for np_ (64 or 128) stacked rows in one go
                    t1 = zp.tile([128, 512], BF16, tag="rt1", name="rt1", bufs=3)
                    t2 = zp.tile([128, 512], BF16, tag="rt2", name="rt2", bufs=3)
                    nc.vector.tensor_mul(
                        t1[0:np_, 0:n], rawt[p0:p0 + np_, off:off + n],
                        cos_sb[p0:p0 + np_, off:off + n])
                    for hb in range(0, np_, 64):
                        for bi in range(2):
                            src = p0 + hb + bi * 32 + (32 if bi % 2 == 0 else -32)
                            nc.vector.tensor_mul(
                                t2[hb + bi * 32:hb + (bi + 1) * 32, 0:n],
                                rawt[src:src + 32, off:off + n],
                                sin_sb[src:src + 32, off:off + n])
                    return t1, t2

                def k_adds(t1, t2, hb, off, n, outt):
                    # even 128-chunks -> outt[0:64], odd -> outt[64:128]
                    # (row-tiled score pairs read odd chunks at base 64)
                    if not ROWTILE_SCORES:
                        nc.vector.tensor_add(
                            outt[0:64, off:off + n],
                            t1[hb:hb + 64, 0:n], t2[hb:hb + 64, 0:n])
                        return
                    if n == 512:
                        def v(ap, base):
                            return ap.rearrange(
                                "p (a b c) -> p a b c", a=2, b=2)[:, :, base, :]
                        for par, dst in ((0, 0), (1, 64)):
                            nc.vector.tensor_add(
                                v(outt[dst:dst + 64, off:off + 512], par),
                                v(t1[hb:hb + 64, 0:512], par),
                                v(t2[hb:hb + 64, 0:512], par))
                    else:
                        lo = 0
                        for i0 in range(off // 128, (off + n + 127) // 128):
                            m = min(128, off + n - i0 * 128)
                            dst = 64 * (i0 % 2)
                            nc.vector.tensor_add(
                                outt[dst:dst + 64, i0 * 128:i0 * 128 + m],
                                t1[hb:hb + 64, lo:lo + m],
                                t2[hb:hb + 64, lo:lo + m])
                            lo += m

                def q_adds(t1, t2, hb, off, n, outt):
                    nc.vector.tensor_add(
                        outt[0:64, off:off + n],
                        t1[hb:hb + 64, 0:n], t2[hb:hb + 64, 0:n])
                    if ROWTILE_SCORES:
                        nc.vector.tensor_copy(
                            outt[64:128, off:off + n], outt[0:64, off:off + n])

                def ropek01(off, n):
                    t1, t2 = rope_mul(rk01, 0, 128, ck_sb, sk_sb, off, n)
                    k_adds(t1, t2, 0, off, n, kt_h[0])
                    k_adds(t1, t2, 64, off, n, kt_h[1])

                def ropeq01(off, n):
                    t1, t2 = rope_mul(rq01, 0, 128, cq_sb, sq_sb, off, n)
                    q_adds(t1, t2, 0, off, n, qt_h[0])
                    q_adds(t1, t2, 64, off, n, qt_h[1])

                def ropek2(off, n):
                    t1, t2 = rope_mul(raw2, 0, 64, ck_sb, sk_sb, off, n)
                    k_adds(t1, t2, 0, off, n, kt_h[2])

                def ropeq2(off, n):
                    t1, t2 = rope_mul(raw2, 64, 64, cq_sb, sq_sb, off, n)
                    q_adds(t1, t2, 0, off, n, qt_h[2])

                jobs = [
                    ("k01", (0, 512)), ("rk01", (0, 512)),
                    ("kq2", (0, 512)), ("rk2", (0, 512)), ("rq2", (0, 512)),
                    ("q01", (0, 512)), ("rq01", (0, 512)),
                    ("v", 0), ("v", 1), ("v", 2), ("v", 3),
                    ("k01", (512, 512)), ("rk01", (512, 512)),
                    ("kq2", (512, 392)),
                    ("q01", (512, 392)), ("rq01", (512, 392)),
                    ("v", 4), ("v", 5),
                    ("k01", (1024, 256)),
                    ("kq2", (904, 256)), ("rk2", (512, 512)),
                    ("rq2", (512, 392)), ("rq2", (904, 256)),
                    ("q01", (904, 256)), ("rq01", (904, 256)),
                    ("k2a", (1160, 120)),
                    ("v", 6), ("v", 7), ("v", 8), ("v", 9),
                    # slice C: h0/h1 K tiles (and their rope) first so the
                    # attention entry is never rope-blocked
                    ("k01", (1280, 512)), ("rk01", (1024, 512)),
                    ("k01", (1792, 512)), ("rk01", (1536, 512)),
                    ("k01", (2304, 16)), ("rk01", (2048, 272)),
                    ("k2a", (1280, 512)), ("rk2", (1024, 512)),
                    ("v", 10), ("v", 11), ("v", 12),
                    ("k2a", (1792, 512)), ("rk2", (1536, 512)),
                    ("v", 13), ("v", 14), ("v", 15),
                    ("k2a", (2304, 16)), ("rk2", (2048, 272)),
                    ("v", 16), ("v", 17), ("v", 18),
                ]
                for kind, arg in jobs:
                    if kind == "v":
                        v_tile(arg)
                    elif kind == "k01":
                        mm128(wk_sb, 0, rk01, *arg)
                    elif kind == "q01":
                        mm128(wq_sb, 1, rq01, *arg)
                    elif kind == "kq2":
                        kq2(*arg)
                    elif kind == "k2a":
                        k2a(*arg)
                    elif kind == "rk01":
                        ropek01(*arg)
                    elif kind == "rq01":
                        ropeq01(*arg)
                    elif kind == "rk2":
                        ropek2(*arg)
                    elif kind == "rq2":
                        ropeq2(*arg)

            # ---- attention ----
            with tc.tile_pool(name="ep", bufs=3) as ep, \
                 tc.tile_pool(name="op", bufs=2) as op, \
                 tc.tile_pool(name="rzp", bufs=2) as rzp, \
                 tc.tile_pool(name="psQ", bufs=1, space="PSUM") as psQ, \
                 tc.tile_pool(name="psP", bufs=1, space="PSUM") as psP, \
                 tc.tile_pool(name="pc3", bufs=1, space="PSUM") as pc3, \
                 tc.tile_pool(name="pp3", bufs=1, space="PSUM") as pp3:
                PROJ_OF_LT = {0: PT[0:4], 1: PT[4:7], 2: PT[7:10]}
                CTX_OF_H = [(ctx01, 0), (ctx01, 64), (ctx2, 0)]

                def proj_slice(toff, tm):
                    outsb = op.tile([128, E], BF16, tag="outsb", name="outsb")
                    for half in range(2):
                        hs = half * 384
                        pp = pp3.tile([128, 512], F32, tag="pp", name="pp")
                        nc.tensor.matmul(
                            pp[0:tm, 0:384], ctx01[0:128, toff:toff + tm],
                            wp1_sb[0:128, hs:hs + 384], start=True, stop=False)
                        nc.tensor.matmul(
                            pp[0:tm, 0:384], ctx2[0:64, toff:toff + tm],
                            wp2_sb[0:64, hs:hs + 384], start=False, stop=True)
                        nc.vector.tensor_copy(outsb[0:tm, hs:hs + 384], pp[0:tm, 0:384])
                    nc.sync.dma_start(out_d[toff:toff + tm, :], outsb[0:tm, :])

                pending = []       # deferred normalize jobs
                proj_ready = []    # proj slices whose ctx inputs are complete

                def finish_tile(z):
                    # NOTE: custom DVE ops (reciprocal_approx_*) must read
                    # and write at the SAME partition base on hardware - the
                    # Z row is parked at base 0 (zrow) before the recip.
                    zrow, ctxu, ctxap, hb, loff2, ln2 = z
                    rzf = zp.tile([1, 512], F32, tag="rzf", name="rzf")
                    nc.vector.reciprocal_approx_fast(rzf[0:1, 0:ln2], zrow[0:1, 0:ln2])
                    rzr = zp.tile([1, 512], F32R, tag="rzr", name="rzr")
                    nc.vector.tensor_copy(rzr[0:1, 0:ln2], rzf[0:1, 0:ln2])
                    przb = pp3.tile([128, 512], F32, tag="pp", name="przb")
                    nc.tensor.matmul(
                        przb[0:64, 0:ln2], ones64[:], rzr[0:1, 0:ln2],
                        start=True, stop=True)
                    nc.vector.tensor_mul(
                        ctxap[hb:hb + 64, loff2:loff2 + ln2],
                        ctxu[0:64, 0:ln2], przb[0:64, 0:ln2])

                def pop_finish():
                    z = pending.pop(0)
                    finish_tile(z)
                    if z[2] is ctx2:
                        proj_ready.extend(PROJ_OF_LT[L_TILES.index((z[4], z[5]))])

                for lt_i, (loff, ln) in enumerate(L_TILES):
                    for h in range(HG):
                        ktap, qtap = kt_h[h], qt_h[h]
                        pctx = pc3.tile([65, 512], F32, tag="pctx")
                        exs = {}
                        if (lt_i * HG + h) % 2 == 0:
                            GROUPS, G_TILE = GROUPS_A, G_TILE_A
                        else:
                            GROUPS, G_TILE = GROUPS_B, G_TILE_B

                        def scores_exp(g, GROUPS=GROUPS, G_TILE=G_TILE):
                            chunks = GROUPS[g]
                            if G_TILE[g] == 'Q':
                                ps = psQ.tile([128, 2048], F32, tag="psq", name="psq")
                            else:
                                ps = psP.tile([128, 1024], F32, tag="psp", name="psp")
                            for j, i in enumerate(chunks):
                                moff, m = MC[i]
                                # row group by CHUNK parity: even chunks live
                                # at kt rows 0:64, odd at rows 64:128
                                hb = 64 * (i % 2) if ROWTILE_SCORES else 0
                                nc.tensor.matmul(
                                    ps[0:m, j * 512:j * 512 + ln],
                                    ktap[hb:hb + 64, moff:moff + m],
                                    qtap[hb:hb + 64, loff:loff + ln],
                                    start=True, stop=True)
                            ex = ep.tile([128, 2048], BF16, tag="ex", name="ex")
                            gk = len(chunks)
                            m0 = MC[chunks[0]][1]
                            if gk > 1:
                                nc.scalar.activation(
                                    ex[0:m0, 0:gk * ln].rearrange(
                                        "p (g n) -> p g n", g=gk),
                                    ps[0:m0, 0:gk * 512].rearrange(
                                        "p (g n) -> p g n", g=gk)[:, :, 0:ln],
                                    AF.Exp, bias=0.0, scale=SCALE)
                            else:
                                nc.scalar.activation(
                                    ex[0:m0, 0:ln], ps[0:m0, 0:ln], AF.Exp,
                                    bias=0.0, scale=SCALE)
                            exs[g] = ex

                        def ctx_mm(g, GROUPS=GROUPS):
                            ex = exs.pop(g)
                            for j, i in enumerate(GROUPS[g]):
                                moff, m = MC[i]
                                nc.tensor.matmul(
                                    pctx[:, 0:ln], v_all[0:m, i, h, :],
                                    ex[0:m, j * ln:j * ln + ln],
                                    start=(i == 0), stop=(i == len(MC) - 1))

                        for g in range(len(GROUPS) + 2):
                            if g < len(GROUPS):
                                scores_exp(g)
                            if g == 2 and pending:
                                pop_finish()
                            if g in (3, 4, 5) and proj_ready:
                                proj_slice(*proj_ready.pop(0))
                            if g >= 2:
                                ctx_mm(g - 2)

                        # park ctx rows + Z row (Z re-based to partition 0);
                        # defer normalize into the next tile's stream
                        zrow = zp.tile([1, 512], F32, tag="zrow", name="zrow")
                        nc.vector.tensor_copy(zrow[0:1, 0:ln], pctx[64:65, 0:ln])
                        ctxu = rzp.tile([64, 512], F32, tag="zc", name="zc")
                        nc.vector.tensor_copy(ctxu[0:64, 0:ln], pctx[0:64, 0:ln])
                        ctxap, hb = CTX_OF_H[h]
                        pending.append((zrow, ctxu, ctxap, hb, loff, ln))

                while pending:
                    pop_finish()
                while proj_ready:
                    proj_slice(*proj_ready.pop(0))
                if DEBUG_DUMP:
                    nc.sync.dma_start(dbg_rk_d[:], rk01[:])
                    nc.sync.dma_start(dbg_kt_d[:], kt_h[0][0:64, :])
                    nc.sync.dma_start(
                        dbg_v_d[:], v_all.rearrange("p a b c -> p (a b c)"))
                    nc.sync.dma_start(dbg_c_d[:], ctx01[:].bitcast(F32))

    nc.finalize()
    return nc


def _rope_tables():
    dim = D // 2
    freqs = 1.0 / 10000 ** (np.arange(0, dim, 2, dtype=np.float64) / dim)
    t = np.arange(GRID, dtype=np.float64)
    f = np.repeat(np.outer(t, freqs), 2, axis=-1)                  # [48, 32]
    fr = np.broadcast_to(f[:, None, :], (GRID, GRID, dim))
    fc = np.broadcast_to(f[None, :, :], (GRID, GRID, dim))
    full = np.concatenate([fr, fc], axis=-1).reshape(GRID * GRID, D)
    cos = np.ones((SEQ, D), np.float64)
    sin = np.zeros((SEQ, D), np.float64)
    cos[TASK:] = np.cos(full)
    sin[TASK:] = np.sin(full)
    return cos.astype(np.float32), sin.astype(np.float32)


def _signed64(tT):
    # [64, S]: signed sine table at the ROTATED (source) rows, so the rope
    # half-multiplies read both operands at equal partition bases.
    return np.ascontiguousarray(np.vstack([tT[32:64], -tT[0:32]]))


def _core_inputs(x, mask, Wqkv, Wproj, bqkv, cos, sin, g, s):
    import ml_dtypes
    bf = ml_dtypes.bfloat16
    xT = x.T  # [768, 2320]
    q0 = SQ * s
    if s == 0:
        perm = None
        xtp = xT
    else:
        perm = np.concatenate([np.arange(SQ, SEQ), np.arange(0, SQ)])
        xtp = np.concatenate([xT[:, SQ:], xT[:, :SQ]], axis=1)
    mkp = mask.astype(np.float32)
    if perm is not None:
        mkp = mkp[perm]
    xtp = xtp * mkp[None, :]   # fold key/value mask into x (V rows -> 0)
    xt = np.ascontiguousarray(
        xtp.reshape(EC, 128, SEQ).transpose(1, 0, 2)).astype(bf)
    r0 = 192 * g

    def wpack(rows):
        w = Wqkv[rows:rows + 192, :].T  # [768, 192]
        return np.ascontiguousarray(
            w.reshape(EC, 128, 192).transpose(1, 0, 2)).astype(bf)

    wq = wpack(r0)
    wk = wpack(768 + r0)
    wv = wpack(1536 + r0)
    wp1 = np.ascontiguousarray(Wproj[:, r0:r0 + 128].T)
    wp2 = np.ascontiguousarray(Wproj[:, r0 + 128:r0 + 192].T)
    b = np.zeros((128, 3), np.float32)
    b[:, 0] = bqkv[768 + r0:768 + r0 + 128]
    b[:, 1] = bqkv[r0:r0 + 128]
    b[0:64, 2] = bqkv[768 + r0 + 128:768 + r0 + 192]
    b[64:128, 2] = bqkv[r0 + 128:r0 + 192]
    cosT, sinT = cos.T, sin.T  # [64, S]
    cq = np.ascontiguousarray(cosT[:, q0:q0 + SQ])
    sq = np.ascontiguousarray(_signed64(sinT)[:, q0:q0 + SQ])
    ckf = cosT
    skf = _signed64(sinT)
    if perm is not None:
        ckf = ckf[:, perm]
        skf = skf[:, perm]
    mk = mask.astype(np.float32)
    if perm is not None:
        mk = mk[perm]
    mk = np.concatenate([mk, np.zeros(19 * 128 - SEQ, np.float32)])
    mk = np.ascontiguousarray(mk.reshape(19, 128).T)
    return {
        "xta": np.ascontiguousarray(xt[:, :, 0:XA]),
        "xtb": np.ascontiguousarray(xt[:, :, XA:XB]),
        "xtc": np.ascontiguousarray(xt[:, :, XB:SEQ]),
        "wq": wq, "wk": wk, "wv": wv,
        "wp1": wp1, "wp2": wp2, "b": b,
        "cq": cq.astype(bf), "sq": sq.astype(bf),
        "ck": np.ascontiguousarray(ckf).astype(bf),
        "sk": np.ascontiguousarray(skf).astype(bf),
        "mk": np.ascontiguousarray(mk),
    }


def _run(x, mask, Wqkv, bqkv, Wproj, bproj, trace=False):
    global _prog
    from concourse.bass_utils import run_bass_kernel_spmd
    if _prog is None:
        _prog = _build()
    x = np.asarray(x, np.float32)
    mask = np.asarray(mask)
    Wqkv = np.asarray(Wqkv, np.float32)
    bqkv = np.asarray(bqkv, np.float32)
    Wproj = np.asarray(Wproj, np.float32)
    bproj = np.asarray(bproj, np.float32)
    cos, sin = _rope_tables()
    in_maps = [
        _core_inputs(x, mask, Wqkv, Wproj, bqkv, cos, sin, core // 2, core % 2)
        for core in range(8)
    ]
    res = run_bass_kernel_spmd(_prog, in_maps, list(range(8)), trace=trace)
    acc = np.zeros((SEQ, E), np.float64)
    for core in range(8):
        s = core % 2
        acc[SQ * s:SQ * (s + 1)] += res.results[core]["pout"].astype(np.float64)
    bias_row = bproj.astype(np.float64) + Wproj.astype(np.float64) @ \
        bqkv[1536:2304].astype(np.float64)
    acc += bias_row
    return acc.astype(np.float32), res


def kernel(x, mask, Wqkv, bqkv, Wproj, bproj):
    out, _ = _run(x, mask, Wqkv, bqkv, Wproj, bproj, trace=False)
    return out
